# revision 5
# baseline (speedup 1.0000x reference)
"""Trainium2 Bass kernel for nn_MultiHeadQGFDLayer (head-sharded SPMD over 8 cores).

Per core (2 heads): QKV projections, p0 = softmax(QK^T/s), P = softmax(KK^T/s'),
two diffusion steps p <- 0.7*p0 + 0.3*(p @ P), attn = p2 @ V, partial out
projection.  The two 2048^3 matmuls per head dominate; their operands are fp16
(PE streams columns at the same rate for all dtypes, fp16 halves SBUF/HBM
traffic), accumulation is fp32 in PSUM, combines/outputs are fp32.
Transposed lhsT operands are produced with XBAR transpose-DMA loads (fp16).
"""

import math
import sys
from contextlib import ExitStack

import numpy as np

sys.path.insert(0, "/opt/trn_rl_repo")

import concourse.bass as bass  # noqa: E402
import concourse.mybir as mybir  # noqa: E402
import concourse.tile as tile  # noqa: E402
from concourse.bass_utils import run_bass_kernel_spmd  # noqa: E402
from concourse.masks import make_identity  # noqa: E402
from concourse.vector_clock import ScopedClock  # noqa: E402

FP32 = mybir.dt.float32
FP16 = mybir.dt.float16
EXP = mybir.ActivationFunctionType.Exp
X_AXIS = mybir.AxisListType.X
P = 128

N_CORES = 8
N_FULL = 2048
D_FULL = 1024
HD_FULL = 64
H_FULL = 16
HL_FULL = H_FULL // N_CORES  # heads per core
ALPHA = 0.3


def _patch_tile_drain():
    """This walrus build rejects >2 sem waits on one instruction; the Tile
    end-of-kernel drain carries one wait per live semaphore.  Split them into
    single-wait drain instructions."""
    if getattr(tile.TileContext, "_drain_split_patched", False):
        return

    def _drain_and_barrier(self, tick_clock, wait_clock):
        collector = self.nc.sync.drain()
        wait_clock.add_sem_waits(
            collector.ins, ScopedClock({None: tick_clock.global_clock})
        )
        si = collector.ins.sync_info
        waits = list(si.on_wait or []) if si else []
        if len(waits) > 1:
            si.on_wait = waits[:1]
            for w in waits[1:]:
                extra = self.nc.sync.drain()
                esi = extra.ins.sync_info
                if esi is None:
                    extra.ins.sync_info = mybir.SyncInfo(on_wait=[w], on_update=[])
                else:
                    esi.on_wait = [w]
        self.nc.all_engine_barrier()
        assert self.sems is not None
        popped = self.nc._tile_sem_poison_stack.pop()
        assert popped is self._sem_poison
        self.nc.clear_and_free_semaphores(list(self.sems.allocated().values()))
        self.nc.all_engine_barrier()

    tile.TileContext._drain_and_barrier = _drain_and_barrier
    tile.TileContext._drain_split_patched = True


def _split_sync_waits(nc, max_waits=2):
    """This walrus build rejects instructions carrying more than ~2 sem waits.
    Move excess waits onto same-engine nop instructions inserted just before
    the over-limit instruction (safe: producers of those waits never depend on
    this engine's progress past the preceding instruction)."""
    for fn in nc.m.functions:
        for blk in fn.blocks:
            insts = list(blk.instructions)
            out = []
            for inst in insts:
                si = inst.sync_info
                waits = list(si.on_wait or []) if si else []
                if len(waits) > max_waits:
                    excess = waits[: len(waits) - max_waits]
                    si.on_wait = waits[len(excess):]
                    eng = nc.engines[inst.engine]
                    for k in range(0, len(excess), max_waits):
                        chunk = excess[k:k + max_waits]
                        nop_bi = eng.nop(nofuse=True)
                        nop = nop_bi.ins
                        removed = False
                        for b2 in fn.blocks:
                            li = b2.instructions
                            if li and li[-1] is nop:
                                li.pop()
                                removed = True
                                break
                        assert removed, "could not relocate nop"
                        nop.sync_info = mybir.SyncInfo(
                            on_wait=list(chunk), on_update=[]
                        )
                        out.append(nop)
                out.append(inst)
            if len(out) != len(insts):
                blk.instructions[:] = out


def build_program(N=N_FULL, D=D_FULL, HL=HL_FULL, HD=HD_FULL):
    """Build the per-core Bass program (SPMD: same program, per-core inputs)."""
    _patch_tile_drain()

    NT = N // P        # 128-row tiles along tokens
    NE = D // P        # 128-row chunks along embed dim
    FD = 512           # matmul moving-operand free dim (one PSUM bank fp32)
    NJ = N // FD
    ND = D // FD
    HDL = HL * HD      # local channels (= 128 at full size)
    assert HDL == P and N % FD == 0 and D % FD == 0

    s_scores = 1.0 / (math.sqrt(HD) + 1e-8)
    s_sim = 1.0 / math.sqrt(HD)
    C1 = ALPHA / (1.0 - ALPHA)

    nc = bass.Bass("TRN2", target_bir_lowering=False, debug=False)

    xt = nc.dram_tensor("xt", [D, N], FP32, kind="ExternalInput").ap()
    wq = nc.dram_tensor("wq", [D, HDL], FP32, kind="ExternalInput").ap()
    wk = nc.dram_tensor("wk", [D, HDL], FP32, kind="ExternalInput").ap()
    wv = nc.dram_tensor("wv", [D, HDL], FP32, kind="ExternalInput").ap()
    bq = nc.dram_tensor("bq", [HDL, 1], FP32, kind="ExternalInput").ap()
    bk = nc.dram_tensor("bk", [HDL, 1], FP32, kind="ExternalInput").ap()
    bv = nc.dram_tensor("bv", [HDL, 1], FP32, kind="ExternalInput").ap()
    wo = nc.dram_tensor("wo", [HDL, D], FP32, kind="ExternalInput").ap()

    p_out = nc.dram_tensor("p_out", [HL, N, N], FP32, kind="ExternalOutput").ap()
    out_part = nc.dram_tensor("out_part", [N, D], FP32, kind="ExternalOutput").ap()

    with tile.TileContext(nc) as tc, ExitStack() as ctx:
        const = ctx.enter_context(tc.tile_pool(name="const", bufs=1))
        presp = ctx.enter_context(tc.tile_pool(name="presp", bufs=1))
        w32p = ctx.enter_context(tc.tile_pool(name="w32p", bufs=3))
        w16p = ctx.enter_context(tc.tile_pool(name="w16p", bufs=3))
        w16bp = ctx.enter_context(tc.tile_pool(name="w16bp", bufs=2))
        lhsp = ctx.enter_context(tc.tile_pool(name="lhsp", bufs=2))
        smallp = ctx.enter_context(tc.tile_pool(name="smallp", bufs=4))
        psum = ctx.enter_context(tc.tile_pool(name="psum", bufs=2, space="PSUM"))
        dram = ctx.enter_context(tc.tile_pool(name="dram", bufs=1, space="DRAM"))

        p0h = dram.tile([HL, N, N], FP16, tag="p0h")
        p1h = dram.tile([HL, N, N], FP16, tag="p1h")
        p2h = dram.tile([HL, N, N], FP16, tag="p2h")

        ident = const.tile([P, P], FP32, tag="ident")
        make_identity(nc, ident)

        qt = const.tile([HDL, N], FP32, tag="qt")      # Q^T  (channels x tokens)
        kt = const.tile([HDL, N], FP32, tag="kt")      # K^T
        vres = const.tile([P, NT, HDL], FP16, tag="vres")  # V rows (fp16)
        atts = [
            const.tile([HD, N], FP32, tag=f"att{h}", name=f"att{h}")
            for h in range(HL)
        ]
        wos = [
            const.tile([HD, D], FP32, tag=f"wos{h}", name=f"wos{h}")
            for h in range(HL)
        ]
        wq_s = const.tile([P, NE, HDL], FP32, tag="wq_s")
        wk_s = const.tile([P, NE, HDL], FP32, tag="wk_s")
        wv_s = const.tile([P, NE, HDL], FP32, tag="wv_s")
        bq_s = const.tile([HDL, 1], FP32, tag="bq_s")
        bk_s = const.tile([HDL, 1], FP32, tag="bk_s")
        bv_s = const.tile([HDL, 1], FP32, tag="bv_s")

        nc.sync.dma_start(wq_s, wq.rearrange("(ne p) c -> p ne c", p=P))
        nc.sync.dma_start(wk_s, wk.rearrange("(ne p) c -> p ne c", p=P))
        nc.sync.dma_start(wv_s, wv.rearrange("(ne p) c -> p ne c", p=P))
        nc.sync.dma_start(bq_s, bq)
        nc.sync.dma_start(bk_s, bk)
        nc.sync.dma_start(bv_s, bv)
        for h in range(HL):
            nc.sync.dma_start(wos[h], wo[h * HD:(h + 1) * HD, :])

        # ---- QKV projections: T-layout via lhsT = W chunk, rhs = X^T chunk.
        for w_s, b_s, dst in ((wq_s, bq_s, qt), (wk_s, bk_s, kt)):
            ps = psum.tile([P, N], FP32, tag="ps")
            for e in range(NE):
                xe = w32p.tile([P, N], FP32, tag="w32")
                nc.sync.dma_start(xe, xt[e * P:(e + 1) * P, :])
                for j in range(NJ):
                    nc.tensor.matmul(
                        ps[:, j * FD:(j + 1) * FD],
                        w_s[:, e, :],
                        xe[:, j * FD:(j + 1) * FD],
                        start=(e == 0),
                        stop=(e == NE - 1),
                    )
            nc.scalar.add(dst, ps[:HDL, :], b_s)

        # V^T, then PE-transpose into row-layout fp16 V.
        ps = psum.tile([P, N], FP32, tag="ps")
        for e in range(NE):
            xe = w32p.tile([P, N], FP32, tag="w32")
            nc.sync.dma_start(xe, xt[e * P:(e + 1) * P, :])
            for j in range(NJ):
                nc.tensor.matmul(
                    ps[:, j * FD:(j + 1) * FD],
                    wv_s[:, e, :],
                    xe[:, j * FD:(j + 1) * FD],
                    start=(e == 0),
                    stop=(e == NE - 1),
                )
        vt_sb = w32p.tile([P, N], FP32, tag="w32")
        nc.scalar.add(vt_sb[:HDL, :], ps[:HDL, :], bv_s)
        for c in range(NT):
            tp = psum.tile([P, N], FP32, tag="ps")
            nc.tensor.transpose(tp[:, :P], vt_sb[:, c * P:(c + 1) * P], ident)
            nc.vector.tensor_copy(vres[:, c, :], tp[:, :HDL])

        for h in range(HL):
            hs = slice(h * HD, (h + 1) * HD)

            # ---- P = softmax(K K^T * s_sim) rows -> SBUF-resident fp16.
            pres = presp.tile([P, NT, N], FP16, tag="pres")
            for i in range(NT):
                ps = psum.tile([P, N], FP32, tag="ps")
                for j in range(NJ):
                    nc.tensor.matmul(
                        ps[:, j * FD:(j + 1) * FD],
                        kt[hs, i * P:(i + 1) * P],
                        kt[hs, j * FD:(j + 1) * FD],
                        start=True,
                        stop=True,
                    )
                ex = w32p.tile([P, N], FP32, tag="w32")
                sums = smallp.tile([P, NJ], FP32, tag="sums")
                for j in range(NJ):
                    nc.scalar.activation(
                        ex[:, j * FD:(j + 1) * FD],
                        ps[:, j * FD:(j + 1) * FD],
                        EXP,
                        scale=s_sim,
                        accum_out=sums[:, j:j + 1],
                    )
                tot = smallp.tile([P, 1], FP32, tag="tot")
                nc.vector.reduce_sum(tot, sums, axis=X_AXIS)
                rec = smallp.tile([P, 1], FP32, tag="rec")
                nc.vector.reciprocal(rec, tot)
                nc.scalar.mul(pres[:, i, :], ex, rec)

            # ---- p0' = (1-a) * softmax(Q K^T * s_scores) rows -> DRAM fp16.
            for i in range(NT):
                ps = psum.tile([P, N], FP32, tag="ps")
                for j in range(NJ):
                    nc.tensor.matmul(
                        ps[:, j * FD:(j + 1) * FD],
                        qt[hs, i * P:(i + 1) * P],
                        kt[hs, j * FD:(j + 1) * FD],
                        start=True,
                        stop=True,
                    )
                ex = w32p.tile([P, N], FP32, tag="w32")
                sums = smallp.tile([P, NJ], FP32, tag="sums")
                for j in range(NJ):
                    nc.scalar.activation(
                        ex[:, j * FD:(j + 1) * FD],
                        ps[:, j * FD:(j + 1) * FD],
                        EXP,
                        scale=s_scores,
                        accum_out=sums[:, j:j + 1],
                    )
                tot = smallp.tile([P, 1], FP32, tag="tot")
                nc.vector.reduce_sum(tot, sums, axis=X_AXIS)
                rec = smallp.tile([P, 1], FP32, tag="rec")
                nc.vector.reciprocal(rec, tot)
                rec7 = smallp.tile([P, 1], FP32, tag="rec7")
                nc.vector.tensor_scalar_mul(rec7, rec, 1.0 - ALPHA)
                p0t = w16p.tile([P, N], FP16, tag="w16")
                nc.scalar.mul(p0t, ex, rec7)
                nc.sync.dma_start(p0h[h, i * P:(i + 1) * P, :], p0t)

            # ---- M1' = p0' @ P ; p1 = p0' + C1 * M1'  -> DRAM fp16.
            for i in range(NT):
                l0 = lhsp.tile([P, NT, P], FP16, tag="lh")
                nc.sync.dma_start_transpose(l0, p0h[h, i * P:(i + 1) * P, :])
                ps = psum.tile([P, N], FP32, tag="ps")
                for c in range(NT):
                    for j in range(NJ):
                        nc.tensor.matmul(
                            ps[:, j * FD:(j + 1) * FD],
                            l0[:, c, :],
                            pres[:, c, j * FD:(j + 1) * FD],
                            start=(c == 0),
                            stop=(c == NT - 1),
                        )
                t1 = w32p.tile([P, N], FP32, tag="w32")
                nc.scalar.mul(t1, ps, C1)
                p0r = w16bp.tile([P, N], FP16, tag="w16b")
                nc.sync.dma_start(p0r, p0h[h, i * P:(i + 1) * P, :])
                p1t = w16p.tile([P, N], FP16, tag="w16")
                nc.vector.tensor_add(p1t, t1, p0r)
                nc.sync.dma_start(p1h[h, i * P:(i + 1) * P, :], p1t)

            # ---- M2 = p1 @ P ; p2 = p0' + a * M2 -> p_out (fp32) + p2h (fp16).
            for i in range(NT):
                l1 = lhsp.tile([P, NT, P], FP16, tag="lh")
                nc.sync.dma_start_transpose(l1, p1h[h, i * P:(i + 1) * P, :])
                ps = psum.tile([P, N], FP32, tag="ps")
                for c in range(NT):
                    for j in range(NJ):
                        nc.tensor.matmul(
                            ps[:, j * FD:(j + 1) * FD],
                            l1[:, c, :],
                            pres[:, c, j * FD:(j + 1) * FD],
                            start=(c == 0),
                            stop=(c == NT - 1),
                        )
                t2 = w32p.tile([P, N], FP32, tag="w32")
                nc.scalar.mul(t2, ps, ALPHA)
                p0r = w16bp.tile([P, N], FP16, tag="w16b")
                nc.sync.dma_start(p0r, p0h[h, i * P:(i + 1) * P, :])
                p2f = w32p.tile([P, N], FP32, tag="w32")
                nc.vector.tensor_add(p2f, t2, p0r)
                nc.sync.dma_start(p_out[h, i * P:(i + 1) * P, :], p2f)
                p2t = w16p.tile([P, N], FP16, tag="w16")
                nc.vector.tensor_add(p2t, t2, p0r)
                nc.sync.dma_start(p2h[h, i * P:(i + 1) * P, :], p2t)

            # ---- attn^T[h] = V_h^T @ p2^T  (accumulate over token chunks).
            aps = psum.tile([P, N], FP32, tag="ps")
            for c in range(NT):
                r2 = w16p.tile([P, N], FP16, tag="w16")
                nc.sync.dma_start_transpose(r2, p2h[h, :, c * P:(c + 1) * P])
                for j in range(NJ):
                    nc.tensor.matmul(
                        aps[:HD, j * FD:(j + 1) * FD],
                        vres[:, c, hs],
                        r2[:, j * FD:(j + 1) * FD],
                        start=(c == 0),
                        stop=(c == NT - 1),
                    )
            nc.vector.tensor_copy(atts[h], aps[:HD, :])

        # ---- out_part = attn @ Wo_slice  (sum over local heads).
        for i in range(NT):
            pso = psum.tile([P, N], FP32, tag="ps")
            for h in range(HL):
                for j in range(ND):
                    nc.tensor.matmul(
                        pso[:, j * FD:(j + 1) * FD],
                        atts[h][:, i * P:(i + 1) * P],
                        wos[h][:, j * FD:(j + 1) * FD],
                        start=(h == 0),
                        stop=(h == HL - 1),
                    )
            o = w32p.tile([P, N], FP32, tag="w32")
            nc.scalar.copy(o[:, :D], pso[:, :D])
            nc.sync.dma_start(out_part[i * P:(i + 1) * P, :], o[:, :D])

    _split_sync_waits(nc, max_waits=1)
    return nc


_NC_CACHE = {}


def _get_nc(key):
    if key not in _NC_CACHE:
        _NC_CACHE[key] = build_program(*key)
    return _NC_CACHE[key]


def make_in_maps(hidden_states, Wq, bq, Wk, bk, Wv, bv, Wo, bo,
                 n_cores=N_CORES, hl=HL_FULL, hd=HD_FULL):
    xt = np.ascontiguousarray(hidden_states[0].T.astype(np.float32))
    hdl = hl * hd
    maps = []
    for c in range(n_cores):
        cs = slice(c * hdl, (c + 1) * hdl)
        maps.append({
            "xt": xt,
            "wq": np.ascontiguousarray(Wq[:, cs]),
            "wk": np.ascontiguousarray(Wk[:, cs]),
            "wv": np.ascontiguousarray(Wv[:, cs]),
            "bq": np.ascontiguousarray(bq[cs].reshape(hdl, 1)),
            "bk": np.ascontiguousarray(bk[cs].reshape(hdl, 1)),
            "bv": np.ascontiguousarray(bv[cs].reshape(hdl, 1)),
            "wo": np.ascontiguousarray(Wo[cs, :]),
        })
    return maps


def kernel(hidden_states, Wq, bq, Wk, bk, Wv, bv, Wo, bo):
    hidden_states = np.asarray(hidden_states, dtype=np.float32)
    Wq, bq = np.asarray(Wq, np.float32), np.asarray(bq, np.float32)
    Wk, bk = np.asarray(Wk, np.float32), np.asarray(bk, np.float32)
    Wv, bv = np.asarray(Wv, np.float32), np.asarray(bv, np.float32)
    Wo, bo = np.asarray(Wo, np.float32), np.asarray(bo, np.float32)

    B, N, D = hidden_states.shape
    nc = _get_nc((N, D, HL_FULL, HD_FULL))
    in_maps = make_in_maps(hidden_states, Wq, bq, Wk, bk, Wv, bv, Wo, bo)
    res = run_bass_kernel_spmd(nc, in_maps, core_ids=list(range(N_CORES)))

    p = np.empty((1, H_FULL, N, N), np.float32)
    acc = np.zeros((N, D), np.float64)
    for c in range(N_CORES):
        p[0, c * HL_FULL:(c + 1) * HL_FULL] = res.results[c]["p_out"]
        acc += res.results[c]["out_part"].astype(np.float64)
    out = (acc + bo.astype(np.float64)).astype(np.float32)[None]
    return out, p
